# revision 5
# baseline (speedup 1.0000x reference)
"""Trainium2 Bass kernel for nn_NetTransform_38362647888184.

Reference computation (B=8, T=2048, H=512), per batch b:
    x      = (e - min(e_all)) / (max(e_all) - min(e_all))      # global minimax
    K[t,j] = prod(x[j:t])  (t>j), 1 (t==j), 0 (t<j)            # (T, T) lower-tri
    h_agg  = (K @ h) / K.sum(-1, keepdims=True)
    out    = h @ h_agg.T / sqrt(H)                              # (T, T)

Strategy: data-parallel over batch, one NeuronCore per batch element.
Per core, K^T tiles (j on partitions, t free) are built exactly with the DVE
hardware scan:  state = x[t-1]*state + delta(t==j)  — the same multiplication
order as the reference cumprod, so no log/exp approximation.  Tensor-engine
matmuls compute h_aggT = h^T K^T (contracted over j, N=512 chunks of t) and
out = h @ h_aggT (contracted over h); the 1/(rowsum*sqrt(H)) factor is applied
as a column scale on the output.  Matmuls run in float32r (full-rate PE mode).
"""

import numpy as np

B, T, H = 8, 2048, 512
NBLK = T // 128   # 16 row blocks
NCH = T // 512    # 4 column chunks
USE_F32R = True

_CACHE = {}


def _split_multiwaits(nc, mybir, max_waits=1):
    """This walrus build rejects >1 sync-wait per instruction; hoist extras
    onto single-wait EventSemaphore nops emitted just before, same engine."""
    for fn in nc.m.functions:
        for blk in fn.blocks:
            insts = blk.instructions
            out = []
            dirty = False
            for inst in insts:
                si = inst.sync_info
                waits = list(si.on_wait) if si is not None else []
                if len(waits) > max_waits:
                    dirty = True
                    for w in waits[:-max_waits]:
                        out.append(
                            mybir.InstEventSemaphore(
                                name=nc.get_next_instruction_name(),
                                engine=inst.engine,
                                ins=[],
                                outs=[],
                                sync_info=mybir.SyncInfo(on_wait=[w], on_update=[]),
                            )
                        )
                    inst.sync_info = mybir.SyncInfo(
                        on_wait=waits[-max_waits:], on_update=list(si.on_update)
                    )
                out.append(inst)
            if dirty:
                blk.instructions = out


def _build(use_f32r=USE_F32R, nch=NCH):
    import concourse.bass as bass
    import concourse.mybir as mybir
    from concourse.tile import TileContext

    fp32 = mybir.dt.float32
    mmdt = mybir.dt.float32r if use_f32r else fp32
    AL = mybir.AluOpType
    AX = mybir.AxisListType

    nc = bass.Bass()
    ea = nc.dram_tensor("ea", [128, 128], fp32, kind="ExternalInput")
    eb = nc.dram_tensor("eb", [T], fp32, kind="ExternalInput")
    hb = nc.dram_tensor("hb", [T, H], mmdt, kind="ExternalInput")
    hTb = nc.dram_tensor("hTb", [H, T], mmdt, kind="ExternalInput")
    dl = nc.dram_tensor("dl", [128, 4 * 512], fp32, kind="ExternalInput")
    out = nc.dram_tensor("out", [T, T], fp32, kind="ExternalOutput")
    xs_d = nc.dram_tensor("xs_d", [T], fp32)
    rs_d = nc.dram_tensor("rs_d", [NCH, 512], fp32)

    with TileContext(nc) as tc:
        with (
            tc.tile_pool(name="const", bufs=1) as cst,
            tc.tile_pool(name="kt", bufs=20) as ktp,
            tc.tile_pool(name="chain", bufs=2) as chp,
            tc.tile_pool(name="hagg", bufs=2) as hgp,
            tc.tile_pool(name="rsx", bufs=2) as rsp,
            tc.tile_pool(name="outs", bufs=4) as osp,
            tc.tile_pool(name="psA", bufs=1, space="PSUM") as psA,
            tc.tile_pool(name="psB", bufs=2, space="PSUM") as psB,
        ):
            # ---- global min / max of e ----
            e_all = cst.tile([128, 128], fp32)
            nc.gpsimd.dma_start(e_all[:], ea[:])
            mx_c = cst.tile([128, 1], fp32)
            mn_c = cst.tile([128, 1], fp32)
            nc.vector.tensor_reduce(mx_c[:], e_all[:], axis=AX.X, op=AL.max)
            nc.vector.tensor_reduce(mn_c[:], e_all[:], axis=AX.X, op=AL.min)
            nmn_c = cst.tile([128, 1], fp32)
            nc.vector.tensor_scalar_mul(nmn_c[:], mn_c[:], -1.0)
            mx = cst.tile([1, 1], fp32)
            neg_mn = cst.tile([1, 1], fp32)
            nc.gpsimd.tensor_reduce(mx[:], mx_c[:], axis=AX.C, op=AL.max)
            nc.gpsimd.tensor_reduce(neg_mn[:], nmn_c[:], axis=AX.C, op=AL.max)
            rng = cst.tile([1, 1], fp32)
            nc.vector.tensor_add(rng[:], mx[:], neg_mn[:])
            inv = cst.tile([1, 1], fp32)
            nc.vector.reciprocal(inv[:], rng[:])

            # ---- normalized x row, padded with leading 1 (the t=0 factor) ----
            x_raw = cst.tile([1, T], fp32)
            nc.gpsimd.dma_start(x_raw[:], eb[:].unsqueeze(0))
            xn = cst.tile([1, T + 1], fp32)
            nc.vector.memset(xn[0:1, 0:1], 1.0)
            nc.vector.tensor_scalar(
                xn[0:1, 1 : T + 1], x_raw[:], neg_mn[0:1, 0:1], inv[0:1, 0:1],
                AL.add, AL.mult,
            )
            # broadcast x (first T entries) down all partitions via DRAM
            nc.gpsimd.dma_start(xs_d[:], xn[0:1, 0:T])
            x_bc = cst.tile([128, T], fp32)
            nc.gpsimd.dma_start(x_bc[:], xs_d[:].unsqueeze(0).broadcast_to([128, T]))

            # ---- constants ----
            deltas = cst.tile([128, 4 * 512], fp32)
            nc.gpsimd.dma_start(deltas[:], dl[:])
            zeros = cst.tile([128, 512], fp32)
            nc.vector.memset(zeros[:], 0.0)
            ones_f32 = cst.tile([128, 1], fp32)
            nc.vector.memset(ones_f32[:], 1.0)
            ones_col = cst.tile([128, 1], mmdt)
            nc.vector.tensor_copy(ones_col[:], ones_f32[:])

            # ---- h and h^T resident ----
            hs = cst.tile([128, NBLK, H], mmdt)
            nc.gpsimd.dma_start(hs[:], hb[:].rearrange("(J p) h -> p J h", p=128))
            hTs = cst.tile([128, H // 128, T], mmdt)
            nc.gpsimd.dma_start(hTs[:], hTb[:].rearrange("(k q) t -> q k t", q=128))

            chain = [None] * NBLK
            for c in range(nch):
                jmax = 4 * c + 3
                # ---- phase A: K^T tiles (scan) + h_aggT / rowsum matmuls ----
                acc = [psA.tile([128, 512], fp32, tag=f"acc{m}", name=f"acc{m}") for m in range(4)]
                rs_ps = psA.tile([1, 512], fp32, tag="rs")
                hg = [hgp.tile([128, 512], mmdt, tag=f"hg{m}", name=f"hg{m}") for m in range(4)]
                for J in range(jmax + 1):
                    kt = ktp.tile([128, 512], mmdt, tag="kt")
                    if J >= 4 * c:
                        r = J - 4 * c
                        d1 = deltas[:, r * 512 : (r + 1) * 512]
                        init = 0.0
                    else:
                        d1 = zeros[:]
                        init = chain[J][:, 0:1]
                    nc.vector.tensor_tensor_scan(
                        kt[:], x_bc[:, c * 512 : (c + 1) * 512], d1, init,
                        AL.mult, AL.add,
                    )
                    if c < nch - 1:
                        ch = chp.tile([128, 1], fp32, tag=f"ch{J}")
                        nc.vector.tensor_copy(ch[:], kt[:, 511:512])
                        chain[J] = ch
                    ktr = kt[:]
                    for m in range(4):
                        nc.tensor.matmul(
                            acc[m][:],
                            hs[:, J, m * 128 : (m + 1) * 128],
                            ktr,
                            start=(J == 0), stop=(J == jmax),
                        )
                    nc.tensor.matmul(
                        rs_ps[:], ones_col[:], ktr,
                        start=(J == 0), stop=(J == jmax),
                    )
                for m in range(4):
                    nc.vector.tensor_copy(hg[m][:], acc[m][:])

                # ---- rowsum -> 1/(rs*sqrt(H)), broadcast via DRAM ----
                rss = rsp.tile([1, 512], fp32, tag="rss")
                nc.vector.tensor_scalar_mul(rss[:], rs_ps[:], float(np.sqrt(H)))
                rsr = rsp.tile([1, 512], fp32, tag="rsr")
                nc.vector.reciprocal(rsr[:], rss[:])
                nc.gpsimd.dma_start(rs_d[c], rsr[0:1, :])
                rsb = rsp.tile([128, 512], fp32, tag="rsb")
                nc.gpsimd.dma_start(
                    rsb[:], rs_d[c].unsqueeze(0).broadcast_to([128, 512])
                )

                # ---- phase B: out[:, c-chunk] = h @ h_aggT_c, column-scaled ----
                for I in range(NBLK):
                    ops = psB.tile([128, 512], fp32, tag="outp")
                    for k in range(4):
                        nc.tensor.matmul(
                            ops[:],
                            hTs[:, k, I * 128 : (I + 1) * 128],
                            hg[k][:],
                            start=(k == 0), stop=(k == 3),
                        )
                    ob = osp.tile([128, 512], fp32, tag="outs")
                    nc.vector.tensor_mul(ob[:], ops[:], rsb[:])
                    nc.gpsimd.dma_start(
                        out[I * 128 : (I + 1) * 128, c * 512 : (c + 1) * 512], ob[:]
                    )

    import concourse.mybir as mybir2
    _split_multiwaits(nc, mybir2)
    return nc


def _deltas_host():
    d = np.zeros((128, 4 * 512), dtype=np.float32)
    for r in range(4):
        for jj in range(128):
            tt = 128 * r + jj
            if tt < 512:
                d[jj, r * 512 + tt] = 1.0
    return d


def kernel(e, h, ilens=None, **_unused):
    from concourse.bass_utils import run_bass_kernel_spmd

    e = np.ascontiguousarray(np.asarray(e, dtype=np.float32))
    h = np.ascontiguousarray(np.asarray(h, dtype=np.float32))
    key = "nc"
    if key not in _CACHE:
        _CACHE[key] = _build()
    nc = _CACHE[key]

    ea = e.reshape(128, 128)
    dl = _deltas_host()
    in_maps = []
    for b in range(B):
        in_maps.append(
            {
                "ea": ea,
                "eb": np.ascontiguousarray(e[b, 0]),
                "hb": np.ascontiguousarray(h[b, 0]),
                "hTb": np.ascontiguousarray(h[b, 0].T),
                "dl": dl,
            }
        )
    res = run_bass_kernel_spmd(nc, in_maps, list(range(B)))
    out = np.stack([res.results[i]["out"] for i in range(B)])[:, None]
    return np.ascontiguousarray(out.astype(np.float32))


# revision 6
# speedup vs baseline: 1.2839x; 1.2839x over previous
"""Trainium2 Bass kernel for nn_NetTransform_38362647888184.

Reference computation (B=8, T=2048, H=512), per batch b:
    x      = (e - min(e_all)) / (max(e_all) - min(e_all))      # global minimax
    K[t,j] = prod(x[j:t])  (t>j), 1 (t==j), 0 (t<j)            # (T, T) lower-tri
    h_agg  = (K @ h) / K.sum(-1, keepdims=True)
    out    = h @ h_agg.T / sqrt(H)                              # (T, T)

Strategy: data-parallel over batch, one NeuronCore per batch element.
K is never materialized: both K@h and K.sum(-1) are first-order linear
recurrences along t —
    h_aggT[:, t] = x[t-1] * h_aggT[:, t-1] + hT[:, t]
    rowsum[t]    = x[t-1] * rowsum[t-1]    + 1
— computed exactly with the DVE hardware scan (state = d0*state + d1), the
same multiplication order as the reference cumprod.  The only tensor-engine
work is the final dense out = h @ h_aggT (contracted over H in 128-blocks),
with the 1/(rowsum*sqrt(H)) factor applied as a column scale on the output.
Matmuls run in float32r (full-rate PE mode).
"""

import numpy as np

B, T, H = 8, 2048, 512
NBLK = T // 128   # 16 row blocks
NCH = T // 512    # 4 column chunks
NKB = H // 128    # 4 h-blocks
USE_F32R = True

_CACHE = {}


def _split_multiwaits(nc, mybir, max_waits=1):
    """This walrus build rejects >1 sync-wait per instruction; hoist extras
    onto single-wait EventSemaphore nops emitted just before, same engine."""
    for fn in nc.m.functions:
        for blk in fn.blocks:
            insts = blk.instructions
            out = []
            dirty = False
            for inst in insts:
                si = inst.sync_info
                waits = list(si.on_wait) if si is not None else []
                if len(waits) > max_waits:
                    dirty = True
                    for w in waits[:-max_waits]:
                        out.append(
                            mybir.InstEventSemaphore(
                                name=nc.get_next_instruction_name(),
                                engine=inst.engine,
                                ins=[],
                                outs=[],
                                sync_info=mybir.SyncInfo(on_wait=[w], on_update=[]),
                            )
                        )
                    inst.sync_info = mybir.SyncInfo(
                        on_wait=waits[-max_waits:], on_update=list(si.on_update)
                    )
                out.append(inst)
            if dirty:
                blk.instructions = out


def _build(use_f32r=USE_F32R, reps=1):
    import concourse.bass as bass
    import concourse.mybir as mybir
    from concourse.tile import TileContext

    fp32 = mybir.dt.float32
    mmdt = mybir.dt.float32r if use_f32r else fp32
    AL = mybir.AluOpType
    AX = mybir.AxisListType

    nc = bass.Bass()
    ea = nc.dram_tensor("ea", [128, 128], fp32, kind="ExternalInput")
    eb = nc.dram_tensor("eb", [T], fp32, kind="ExternalInput")
    hTb = nc.dram_tensor("hTb", [H, T], mmdt, kind="ExternalInput")
    out = nc.dram_tensor("out", [T, T], fp32, kind="ExternalOutput")
    xs_d = nc.dram_tensor("xs_d", [T], fp32)
    rs_d = nc.dram_tensor("rs_d", [T], fp32)

    with TileContext(nc) as tc:
        with (
            tc.tile_pool(name="const", bufs=1) as cst,
            tc.tile_pool(name="hagg", bufs=2) as hgp,
            tc.tile_pool(name="outs", bufs=4) as osp,
            tc.tile_pool(name="psB", bufs=4, space="PSUM") as psB,
        ):
            # ---- global min / max of e ----
            e_all = cst.tile([128, 128], fp32)
            nc.gpsimd.dma_start(e_all[:], ea[:])
            mx_c = cst.tile([128, 1], fp32)
            mn_c = cst.tile([128, 1], fp32)
            nc.vector.tensor_reduce(mx_c[:], e_all[:], axis=AX.X, op=AL.max)
            nc.vector.tensor_reduce(mn_c[:], e_all[:], axis=AX.X, op=AL.min)
            nmn_c = cst.tile([128, 1], fp32)
            nc.vector.tensor_scalar_mul(nmn_c[:], mn_c[:], -1.0)
            mx = cst.tile([1, 1], fp32)
            neg_mn = cst.tile([1, 1], fp32)
            nc.gpsimd.tensor_reduce(mx[:], mx_c[:], axis=AX.C, op=AL.max)
            nc.gpsimd.tensor_reduce(neg_mn[:], nmn_c[:], axis=AX.C, op=AL.max)
            rng = cst.tile([1, 1], fp32)
            nc.vector.tensor_add(rng[:], mx[:], neg_mn[:])
            inv = cst.tile([1, 1], fp32)
            nc.vector.reciprocal(inv[:], rng[:])

            # ---- normalized x row, padded with leading 1 (the t=0 factor) ----
            x_raw = cst.tile([1, T], fp32)
            nc.gpsimd.dma_start(x_raw[:], eb[:].unsqueeze(0))
            xn = cst.tile([1, T + 1], fp32)
            nc.vector.memset(xn[0:1, 0:1], 1.0)
            nc.vector.tensor_scalar(
                xn[0:1, 1 : T + 1], x_raw[:], neg_mn[0:1, 0:1], inv[0:1, 0:1],
                AL.add, AL.mult,
            )
            # broadcast x[t-1] (first T entries of xn) down all partitions
            nc.gpsimd.dma_start(xs_d[:], xn[0:1, 0:T])
            x_bc = cst.tile([128, T], fp32)
            nc.gpsimd.dma_start(x_bc[:], xs_d[:].unsqueeze(0).broadcast_to([128, T]))

            # ---- rowsum scan: rs[t] = x[t-1]*rs[t-1] + 1, rs[0] = 1 ----
            ones_row = cst.tile([1, T], fp32)
            nc.vector.memset(ones_row[:], 1.0)
            rs_row = cst.tile([1, T], fp32)
            nc.vector.tensor_tensor_scan(
                rs_row[:], xn[0:1, 0:T], ones_row[:], 0.0, AL.mult, AL.add
            )
            rss = cst.tile([1, T], fp32)
            nc.vector.tensor_scalar_mul(rss[:], rs_row[:], float(np.sqrt(H)))
            rsr = cst.tile([1, T], fp32)
            nc.vector.reciprocal(rsr[:], rss[:])
            nc.gpsimd.dma_start(rs_d[:], rsr[0:1, :])
            rsb = cst.tile([128, T], fp32)
            nc.gpsimd.dma_start(rsb[:], rs_d[:].unsqueeze(0).broadcast_to([128, T]))

            # ---- h^T resident: partitions = h (4 blocks), free = t ----
            hTs = cst.tile([128, NKB, T], mmdt)
            nc.gpsimd.dma_start(hTs[:], hTb[:].rearrange("(k q) t -> q k t", q=128))

            for _rep in range(reps):
                hgprev = [None] * NKB
                for c in range(NCH):
                    lo = c * 512
                    # h_aggT chunk via scan; chain initial from previous chunk
                    hg = [
                        hgp.tile([128, 512], mmdt, tag=f"hg{k}", name=f"hg{k}")
                        for k in range(NKB)
                    ]
                    for k in range(NKB):
                        init = 0.0 if c == 0 else hgprev[k][:, 511:512]
                        nc.vector.tensor_tensor_scan(
                            hg[k][:],
                            x_bc[:, lo : lo + 512],
                            hTs[:, k, lo : lo + 512],
                            init,
                            AL.mult, AL.add,
                        )
                    hgprev = hg
                    # out[:, chunk c] = h @ h_aggT_c, column-scaled
                    for I in range(NBLK):
                        ops = psB.tile([128, 512], fp32, tag="outp")
                        for k in range(NKB):
                            nc.tensor.matmul(
                                ops[:],
                                hTs[:, k, I * 128 : (I + 1) * 128],
                                hg[k][:],
                                start=(k == 0), stop=(k == NKB - 1),
                            )
                        ob = osp.tile([128, 512], fp32, tag="outs")
                        nc.vector.tensor_mul(ob[:], ops[:], rsb[:, lo : lo + 512])
                        nc.gpsimd.dma_start(
                            out[I * 128 : (I + 1) * 128, lo : lo + 512], ob[:]
                        )

    import concourse.mybir as mybir2
    _split_multiwaits(nc, mybir2)
    return nc


def kernel(e, h, ilens=None, **_unused):
    from concourse.bass_utils import run_bass_kernel_spmd

    e = np.ascontiguousarray(np.asarray(e, dtype=np.float32))
    h = np.ascontiguousarray(np.asarray(h, dtype=np.float32))
    if "nc" not in _CACHE:
        _CACHE["nc"] = _build()
    nc = _CACHE["nc"]

    ea = e.reshape(128, 128)
    in_maps = []
    for b in range(B):
        in_maps.append(
            {
                "ea": ea,
                "eb": np.ascontiguousarray(e[b, 0]),
                "hTb": np.ascontiguousarray(h[b, 0].T),
            }
        )
    res = run_bass_kernel_spmd(nc, in_maps, list(range(B)))
    out = np.stack([res.results[i]["out"] for i in range(B)])[:, None]
    return np.ascontiguousarray(out.astype(np.float32))
